# revision 17
# baseline (speedup 1.0000x reference)
"""Trainium2 Bass kernel for nn_PairwiseAttentionTerminal (v2).

Reference computation (L=1024, B=8, F=256, H=8, C=32):
    x = layernorm(features)                       # (L, B, F)
    q,k,v = x@Wq+bq, x@Wk+bk, x@Wv+bv             # (L, B, H, C)
    bias  = x@Wb+bb                               # (L, B, H) per-key bias
    gate  = sigmoid(x@Wg+bg)                      # (L, B, H, C)
    S     = einsum('qbhc,kbhc->qbkh', q, k)/sqrt(C) + bias[None]
    attn  = softmax_k(S) @ v                      # (L, B, H, C)
    out   = (attn*gate) @ Wo + bo                 # (L, B, F)

Sharding: batch B=8 -> one batch element per NeuronCore, weights replicated,
no collectives.

Design (cost-model-driven):
  - LN stats via bn_stats/bn_aggr (DVE); xn bf16 on GPSIMD; transpose via
    XBAR DMA (dma_start_transpose) -> no PE transposes, no PSUM copies.
  - All projections as fp8e4 DoubleRow matmuls (contraction 256 = 128x2).
  - S^T per (head, k-tile) as fp8-DR matmul with a stride-0 broadcast pair
    dim (computes 2x the true product; folded into exp scale).
  - Per-key softmax bias folded into V: vaug = 16*exp(bT)*v', plus
    replicated denominator columns (M=64/head: even h [attn|den] at rows
    0:64 of the pair psum, odd h [den|attn] at 64:128) -> den rows 32:96
    contiguous: ONE reciprocal_approx_fast per head pair.
  - exp split between ACT (native Exp) and DVE (custom EXP2_SQ16_ANT:
    quadratic + 4 squarings, one uop, <1e-3 rel err), writing fp8 eT2
    tiles [128,2,512] = the DoubleRow rhs of the AV matmul.
  - gate = 1/(1+exp(-y)) via ACT exp + DVE add1 + reciprocal_approx_fast.
  - engine split: GPSIMD takes xn, xT8 convert, vaug ones, final gate-mult
    (all SBUF-only; GPSIMD has no PSUM port). Single act table (Ln+Exp).
"""

import numpy as np
from contextlib import ExitStack

L, B, F, H, C = 1024, 8, 256, 8, 32
HC = H * C
EPS = 1e-5
N_CORES = 8
P = 128
NLT = L // P   # 8 L-tiles
QSC = 8.0      # q/k fp8 pre-scale (folded into Wq/Wk host-side)
S_SCALE = 0.0013810679320049755   # 1/(2*QSC^2*sqrt(C)): psum -> logits
# EXP2_SQ16 constants: out = exp(x*S_SCALE), fit for |logits| <= 0.8
EXP_C0 = 6.103612520034756e-05
EXP_C1 = 0.7073182017382096
EXP_C2 = 0.4997011001858717
VSC = 16.0     # vaug scale (cancels in normalize)

_COMPILED = {}
_EXP2_OP = None


def _register_exp2():
    """Register the custom DVE op EXP2_SQ16_ANT (idempotent)."""
    global _EXP2_OP
    if _EXP2_OP is not None:
        return _EXP2_OP
    import concourse.dve_ops as dvo
    import concourse.dve_spec as dvs
    from concourse.dve_spec import Src0, C0, C1, C2, Spec, sq
    import concourse.dve_uop as dvu

    for op in dvo.OPS:
        if op.name == "EXP2_SQ16_ANT":
            _EXP2_OP = op
            return op

    _body = sq(sq(sq(sq(sq(Src0 * C0 + C1) + C2))))

    def _ref(in0, in1, c0, c1, c2):
        z = in0.astype(np.float32) * np.float32(c0)
        q = (z + np.float32(c1)) * (z + np.float32(c1)) + np.float32(c2)
        for _ in range(4):
            q = q * q
        return q

    spec = Spec(body=_body, reference=_ref)
    row = dvo._CUSTOM_DVE_ROW_BASE + len(dvo.OPS)
    assert row < 0x20, "custom-DVE opcode rows exhausted"
    shas = {}
    for ver in ("v3", "v4"):
        r = dvu.DveOpSpec(name="EXP2_SQ16_ANT", opcode=row,
                          uops=dvs.lower(spec, ver=ver),
                          rd1_en=dvs._has_src1(spec))
        shas[ver] = r.sha(ver)
    op = dvo.DveOp("EXP2_SQ16_ANT", spec, subdim=False, uops_sha=shas)
    dvo.OPS.append(op)
    dvo.CUSTOM_DVE_SPECS[op.name] = op.spec
    dvo._SUB_OPCODE_FOR_NAME[op.name] = row
    _EXP2_OP = op
    return op


def _build(bv_zero=True, bo_zero=True, bqk_zero=True, dve_exp_every=8):
    import concourse.bacc as bacc
    import concourse.mybir as mybir
    import concourse.tile as tile
    from concourse.dve_ops import RECIP_APPROX_FAST_CONSTS, RECIPROCAL_APPROX_FAST

    exp2op = _register_exp2()

    f32 = mybir.dt.float32
    bf16 = mybir.dt.bfloat16
    fp8 = mybir.dt.float8e4
    u8 = mybir.dt.uint8
    u16 = mybir.dt.uint16
    AF = mybir.ActivationFunctionType
    ALU = mybir.AluOpType
    DR = mybir.MatmulPerfMode.DoubleRow
    RCN = RECIP_APPROX_FAST_CONSTS

    nc = bacc.Bacc("TRN2", target_bir_lowering=False)

    feat_e = nc.dram_tensor("feat", [L, F], f32, kind="ExternalInput")
    wq_e = nc.dram_tensor("wq16", [P, 2, HC], u16, kind="ExternalInput")
    wk_e = nc.dram_tensor("wk16", [P, 2, HC], u16, kind="ExternalInput")
    wg_e = nc.dram_tensor("wg16", [P, 2, HC], u16, kind="ExternalInput")
    wvb_e = nc.dram_tensor("wvb16", [P, 2, HC + H], u16, kind="ExternalInput")
    wo_e = nc.dram_tensor("wo16", [P, 2, F], u16, kind="ExternalInput")
    bq_e = nc.dram_tensor("bq_t", [P, 2], f32, kind="ExternalInput")
    bk_e = nc.dram_tensor("bk_t", [P, 2], f32, kind="ExternalInput")
    nbg_e = nc.dram_tensor("nbg_t", [P, 2], f32, kind="ExternalInput")
    bv_e = nc.dram_tensor("bv_t", [P, 2], f32, kind="ExternalInput")
    bo_e = nc.dram_tensor("bo_b", [P, F], f32, kind="ExternalInput")
    out_e = nc.dram_tensor("out", [L, F], f32, kind="ExternalOutput")

    with tile.TileContext(nc) as tc, ExitStack() as ctx:
        const = ctx.enter_context(tc.tile_pool(name="const", bufs=1))
        main = ctx.enter_context(tc.tile_pool(name="main", bufs=1))
        work = ctx.enter_context(tc.tile_pool(name="work", bufs=4))
        epool = ctx.enter_context(tc.tile_pool(name="epool", bufs=4))
        npool = ctx.enter_context(tc.tile_pool(name="npool", bufs=4))
        opool = ctx.enter_context(tc.tile_pool(name="opool", bufs=4))

        ftp = ctx.enter_context(tc.tile_pool(name="ftp", bufs=1))
        ft = [ftp.tile([P, F], f32, name=f"ft{i}") for i in range(NLT)]
        for i in range(NLT):
            (nc.sync if i % 2 == 0 else nc.gpsimd).dma_start(
                ft[i][:], feat_e.ap()[i * P:(i + 1) * P, :])

        def load(name, ext, shape, dt_):
            t = const.tile(shape, dt_, name=name)
            nc.sync.dma_start(t[:], ext.ap().bitcast(dt_))
            return t

        wq16 = load("wq16_s", wq_e, [P, 2, HC], bf16)
        wk16 = load("wk16_s", wk_e, [P, 2, HC], bf16)
        wvb16 = load("wvb16_s", wvb_e, [P, 2, HC + H], bf16)
        epst = const.tile([P, 1], f32, name="epst")
        nc.vector.memset(epst[:], EPS)
        ln16t = const.tile([P, 1], f32, name="ln16t")
        nc.vector.memset(ln16t[:], float(np.log(VSC)))
        dumt = const.tile([P, 1], f32, name="dumt")
        nc.scalar.activation(dumt[:], epst[:], AF.Exp)

        xT16 = [main.tile([P, L], bf16, name=f"xT16_{j}") for j in range(2)]
        q8 = [main.tile([P, L], fp8, name=f"q8_{j}") for j in range(2)]
        k8 = [main.tile([P, L], fp8, name=f"k8_{j}") for j in range(2)]
        g16 = [main.tile([P, L], bf16, name=f"g16_{j}") for j in range(2)]
        agT = [main.tile([P, L], bf16, name=f"agT{j}") for j in range(2)]
        v16 = main.tile([P, NLT, H, 64], bf16, name="v16")
        ebT = [main.tile([P, H], f32, name=f"ebT{i}") for i in range(NLT)]

        # ================= Stage A: LN + DMA transpose =================
        for i in range(NLT):
            st6 = work.tile([P, 6], f32, tag="st6")
            nc.vector.bn_stats(st6[:], ft[i][:])
            ag = work.tile([P, 4], f32, tag="ag")
            nc.vector.bn_aggr(ag[:, 0:2], st6[:])
            nc.scalar.activation(ag[:, 2:3], ag[:, 1:2], AF.Ln, bias=epst[:])
            nc.scalar.activation(ag[:, 3:4], ag[:, 2:3], AF.Exp, scale=-0.5)
            xn = work.tile([P, F], bf16, tag="xn")
            nc.vector.tensor_scalar(xn[:], ft[i][:], ag[:, 0:1], ag[:, 3:4],
                                    op0=ALU.subtract, op1=ALU.mult)
            ls = slice(i * P, (i + 1) * P)
            for jc in range(2):
                nc.sync.dma_start_transpose(xT16[jc][:, ls],
                                            xn[:, jc * P:(jc + 1) * P])

        wg16 = load("wg16_s", wg_e, [P, 2, HC], bf16)
        wo16 = load("wo16_s", wo_e, [P, 2, F], bf16)
        bq4 = load("bq4_s", bq_e, [P, 2], f32)
        bk4 = load("bk4_s", bk_e, [P, 2], f32)
        nbg = load("nbg_s", nbg_e, [P, 2], f32)
        bvt = load("bvt_s", bv_e, [P, 2], f32)
        bob = load("bob_s", bo_e, [P, F], f32)


        # ================= Stage B: projections =================
        psB_cm = tc.tile_pool(name="psB", bufs=3, space="PSUM")
        psB = psB_cm.__enter__()
        psG_cm = tc.tile_pool(name="psG", bufs=1, space="PSUM")
        psG = psG_cm.__enter__()
        psVp_cm = tc.tile_pool(name="psVp", bufs=2, space="PSUM")
        psVp = psVp_cm.__enter__()

        for jc in range(2):
            cs = slice(jc * P, (jc + 1) * P)
            for m in range(2):
                ms = slice(512 * m, 512 * (m + 1))
                pq = psB.tile([P, 512], f32, tag="pb", name=f"pq{jc}{m}")
                pk = psB.tile([P, 512], f32, tag="pb", name=f"pk{jc}{m}")
                for fc in range(2):
                    nc.tensor.matmul(pq[:], wq16[:, fc, cs],
                                     xT16[fc][:, ms], start=(fc == 0),
                                     stop=(fc == 1))
                    nc.tensor.matmul(pk[:], wk16[:, fc, cs],
                                     xT16[fc][:, ms], start=(fc == 0),
                                     stop=(fc == 1))
                if bqk_zero:
                    nc.vector.tensor_copy(q8[jc][:, ms], pq[:])
                    nc.vector.tensor_copy(k8[jc][:, ms], pk[:])
                else:
                    nc.vector.tensor_scalar(q8[jc][:, ms], pq[:],
                                            bq4[:, jc:jc + 1], None,
                                            op0=ALU.add)
                    nc.vector.tensor_scalar(k8[jc][:, ms], pk[:],
                                            bk4[:, jc:jc + 1], None,
                                            op0=ALU.add)
            pg = psG.tile([P, L], f32, tag="pg", name=f"pg{jc}")
            for m in range(2):
                ms = slice(512 * m, 512 * (m + 1))
                for fc in range(2):
                    nc.tensor.matmul(pg[:, ms], wg16[:, fc, cs],
                                     xT16[fc][:, ms], start=(fc == 0),
                                     stop=(fc == 1))
            eg = work.tile([P, L], f32, tag="eg", name=f"eg{jc}")
            nc.scalar.activation(eg[:], pg[:], AF.Exp,
                                 bias=nbg[:, jc:jc + 1], scale=-1.0)
            nc.vector.tensor_scalar(eg[:], eg[:], 1.0, None, op0=ALU.add)
            nc.vector._custom_dve(RECIPROCAL_APPROX_FAST, out=g16[jc][:],
                                  in0=eg[:], s0=RCN["s0"], s1=RCN["s1"],
                                  imm2=RCN["imm2"])

        for i in range(NLT):
            ls = slice(i * P, (i + 1) * P)
            pv = psVp.tile([P, HC + H], f32, tag="pv", name=f"pv{i}")
            for fc in range(2):
                nc.tensor.matmul(pv[:], xT16[fc][:, ls], wvb16[:, fc, :],
                                 start=(fc == 0), stop=(fc == 1))
            # eb = 16*exp(bT): fold the vaug scale into the exp bias
            nc.scalar.activation(ebT[i][:], pv[:, HC:HC + H], AF.Exp,
                                 bias=ln16t[:])
            vv = pv[:, 0:HC].rearrange("p (h c) -> p h c", h=H)
            eb = ebT[i][:].unsqueeze(2).to_broadcast([P, H, 32])
            nc.vector.tensor_tensor(v16[:, i, :, 0:32], vv[:], eb,
                                    op=ALU.mult)
            nc.gpsimd.tensor_copy(v16[:, i, :, 32:64], eb)

        psVp_cm.__exit__(None, None, None)
        psG_cm.__exit__(None, None, None)
        psB_cm.__exit__(None, None, None)

        # ================= Stage C: attention =================
        psS_cm = tc.tile_pool(name="psS", bufs=2, space="PSUM")
        psS = psS_cm.__enter__()
        psV_cm = tc.tile_pool(name="psV", bufs=2, space="PSUM")
        psV = psV_cm.__enter__()

        expctr = 0
        for h in range(H):
            jh, hp = h // 4, slice(32 * (h % 4), 32 * (h % 4) + 32)
            aph = psV.tile([64, L], f32, tag="av", name=f"aph{h}")
            for p in range(4):  # k-tile pairs
                for m in range(2):
                    ms = slice(512 * m, 512 * (m + 1))
                    sp = psS.tile([P, L], f32, tag="sp", name=f"sp{h}{p}{m}")
                    for j in range(2):
                        ks = slice((2 * p + j) * P, (2 * p + j + 1) * P)
                        nc.tensor.matmul(
                            sp[:, 512 * j:512 * (j + 1)],
                            k8[jh][hp, ks].unsqueeze(1)
                                .to_broadcast([32, 2, P]),
                            q8[jh][hp, ms].unsqueeze(1)
                                .to_broadcast([32, 2, 512]),
                            start=True, stop=True, perf_mode=DR,
                            tile_position=(32 * (h % 4), 0))
                    eT = epool.tile([P, 2, 512], bf16, tag="eT",
                                    name=f"eT{h}{p}{m}")
                    spv = sp[:].rearrange("a (b c) -> a b c", b=2)
                    expctr += 1
                    if dve_exp_every and expctr % dve_exp_every == 0 \
                            and expctr <= 56:
                        nc.vector._custom_dve(exp2op, out=eT[:], in0=spv,
                                              s0=EXP_C0, s1=EXP_C1,
                                              imm2=EXP_C2)
                    else:
                        nc.scalar.activation(eT[:], spv, AF.Exp,
                                             scale=S_SCALE)
                    for j in range(2):
                        nc.tensor.matmul(aph[:, ms], v16[:, 2 * p + j, h, :],
                                         eT[:, j, :],
                                         start=(p == 0 and j == 0),
                                         stop=(p == 3 and j == 1))
            # drain: nrm = 1/aph (rows 32:64 are the denominator; rows 0:32
            # give unused junk — cost is free-size only), th = attn*nrm,
            # agT = (th [+bv]) * gate on GPSIMD (SBUF-only)
            nrm = npool.tile([64, L], bf16, tag="nrm", name=f"nrm{h}")
            th = npool.tile([P, L], bf16, tag="th", name=f"th{h}")
            msplits = ([slice(0, 512), slice(512, 1024)] if h == H - 1
                       else [slice(0, L)])
            for dms in msplits:
                nc.vector._custom_dve(RECIPROCAL_APPROX_FAST,
                                      out=nrm[:, dms], in0=aph[:, dms],
                                      s0=RCN["s0"], s1=RCN["s1"],
                                      imm2=RCN["imm2"])
                nc.vector.tensor_tensor(th[hp, dms], aph[0:32, dms],
                                        nrm[32:64, dms], op=ALU.mult)
                nc.gpsimd.tensor_tensor(agT[jh][hp, dms], th[hp, dms],
                                        g16[jh][hp, dms], op=ALU.mult)
                if not bv_zero:
                    nc.vector.scalar_tensor_tensor(
                        agT[jh][hp, dms], g16[jh][hp, dms],
                        bvt[hp, jh:jh + 1], agT[jh][hp, dms],
                        op0=ALU.mult, op1=ALU.add)

        psV_cm.__exit__(None, None, None)

        # ================= Stage D: output projection =================
        for i in range(NLT):
            ls = slice(i * P, (i + 1) * P)
            po = psS.tile([P, F], f32, tag="sp", name=f"po{i}")
            nc.tensor.matmul(po[:], agT[0][:, ls], wo16[:, 0, :],
                             start=True, stop=False)
            nc.tensor.matmul(po[:], agT[1][:, ls], wo16[:, 1, :],
                             start=False, stop=True)
            o = opool.tile([P, F], f32, tag="o", name=f"o{i}")
            if bo_zero:
                if i % 2 == 0:
                    nc.vector.tensor_copy(o[:], po[:])
                else:
                    nc.scalar.copy(o[:], po[:])
            else:
                nc.vector.tensor_tensor(o[:], po[:], bob[:], op=ALU.add)
            (nc.sync if i % 2 == 0 else nc.gpsimd).dma_start(
                out_e.ap()[ls, :], o[:])

        psS_cm.__exit__(None, None, None)

    # single activation table (Exp+Ln) to avoid chooser thrash
    import concourse.bacc as bacc_mod
    orig_gat = bacc_mod.get_activation_tables
    AFt = mybir.ActivationFunctionType

    def gat_combined(arch):
        t = orig_gat(arch)
        out = {}
        drop = {AFt.Exp, AFt.Ln}
        for name, funcs in t.items():
            if name == "natural_log_exp_and_others":
                out[name] = funcs
            else:
                out[name] = funcs - drop
        return out

    bacc_mod.get_activation_tables = gat_combined
    try:
        nc.compile()
    finally:
        bacc_mod.get_activation_tables = orig_gat
    return nc


def _prep_inputs(features, ln_g, ln_b, Wq, bq, Wk, bk, Wv, bv, Wb, bb,
                 Wg, bg, Wo, bo):
    import ml_dtypes
    f32 = np.float32
    fp8 = ml_dtypes.float8_e4m3
    bfl = ml_dtypes.bfloat16
    g_ = np.asarray(ln_g, f32)[:, None]
    b_ = np.asarray(ln_b, f32)

    def chunk3(W, scale, dt):
        Wn = (np.asarray(W, f32) * g_ * scale).reshape(2, P, -1)
        return np.ascontiguousarray(Wn.transpose(1, 0, 2)).astype(dt)

    def bchunk(v):
        return np.ascontiguousarray(np.asarray(v, f32).reshape(2, P).T)

    Wq_ = chunk3(Wq, QSC, bfl).view(np.uint16)
    Wk_ = chunk3(Wk, QSC, bfl).view(np.uint16)
    Wg_ = chunk3(Wg, 1.0, bfl).view(np.uint16)
    Wvb = np.concatenate([np.asarray(Wv, f32), np.asarray(Wb, f32)], axis=1)
    Wvb_ = chunk3(Wvb, 1.0, bfl).view(np.uint16)
    Wo_ = np.ascontiguousarray(
        np.asarray(Wo, f32).reshape(2, P, F).transpose(1, 0, 2)
    ).astype(bfl).view(np.uint16)

    bqf = (b_ @ np.asarray(Wq, f32) + np.asarray(bq, f32)) * QSC
    bkf = (b_ @ np.asarray(Wk, f32) + np.asarray(bk, f32)) * QSC
    bgf = b_ @ np.asarray(Wg, f32) + np.asarray(bg, f32)
    bvf = b_ @ np.asarray(Wv, f32) + np.asarray(bv, f32)

    common = {
        "wq16": Wq_, "wk16": Wk_, "wg16": Wg_, "wvb16": Wvb_, "wo16": Wo_,
        "bq_t": bchunk(bqf), "bk_t": bchunk(bkf), "nbg_t": bchunk(-bgf),
        "bv_t": bchunk(bvf),
        "bo_b": np.ascontiguousarray(np.tile(np.asarray(bo, f32), (P, 1))),
    }
    flags = (bool(np.all(bvf == 0.0)), bool(np.all(np.asarray(bo, f32) == 0.0)),
             bool(np.all(bqf == 0.0) and np.all(bkf == 0.0)))
    feats = np.asarray(features, f32)
    in_maps = []
    for b in range(N_CORES):
        m = dict(common)
        m["feat"] = np.ascontiguousarray(feats[:, b, :])
        in_maps.append(m)
    return in_maps, flags


def kernel(**inputs):
    from concourse.bass_utils import run_bass_kernel_spmd

    in_maps, flags = _prep_inputs(**inputs)
    key = ("nc",) + flags
    if key not in _COMPILED:
        bv_zero, bo_zero, bqk_zero = flags
        _COMPILED[key] = _build(bv_zero=bv_zero, bo_zero=bo_zero,
                                bqk_zero=bqk_zero)
        _COMPILED["nc"] = _COMPILED[key]
    nc = _COMPILED[key]
    res = run_bass_kernel_spmd(nc, in_maps, list(range(N_CORES)))
    out = np.stack([res.results[b]["out"] for b in range(N_CORES)], axis=1)
    return np.ascontiguousarray(out.astype(np.float32))


if __name__ == "__main__":
    rng = np.random.default_rng(0)
    ins = {
        "features": rng.standard_normal((L, B, F), dtype=np.float32),
        "ln_g": np.ones(F, np.float32), "ln_b": np.zeros(F, np.float32),
        "Wq": rng.standard_normal((F, HC), dtype=np.float32) * 0.02,
        "bq": np.zeros(HC, np.float32),
        "Wk": rng.standard_normal((F, HC), dtype=np.float32) * 0.02,
        "bk": np.zeros(HC, np.float32),
        "Wv": rng.standard_normal((F, HC), dtype=np.float32) * 0.02,
        "bv": np.zeros(HC, np.float32),
        "Wb": rng.standard_normal((F, H), dtype=np.float32) * 0.02,
        "bb": np.zeros(H, np.float32),
        "Wg": rng.standard_normal((F, HC), dtype=np.float32) * 0.02,
        "bg": np.zeros(HC, np.float32),
        "Wo": rng.standard_normal((HC, F), dtype=np.float32) * 0.02,
        "bo": np.zeros(F, np.float32),
    }
    print(kernel(**ins).shape)


# revision 18
# speedup vs baseline: 1.0069x; 1.0069x over previous
"""Trainium2 Bass kernel for nn_PairwiseAttentionTerminal (v2).

Reference computation (L=1024, B=8, F=256, H=8, C=32):
    x = layernorm(features)                       # (L, B, F)
    q,k,v = x@Wq+bq, x@Wk+bk, x@Wv+bv             # (L, B, H, C)
    bias  = x@Wb+bb                               # (L, B, H) per-key bias
    gate  = sigmoid(x@Wg+bg)                      # (L, B, H, C)
    S     = einsum('qbhc,kbhc->qbkh', q, k)/sqrt(C) + bias[None]
    attn  = softmax_k(S) @ v                      # (L, B, H, C)
    out   = (attn*gate) @ Wo + bo                 # (L, B, F)

Sharding: batch B=8 -> one batch element per NeuronCore, weights replicated,
no collectives.

Design (cost-model-driven):
  - LN stats via bn_stats/bn_aggr (DVE); xn bf16 on GPSIMD; transpose via
    XBAR DMA (dma_start_transpose) -> no PE transposes, no PSUM copies.
  - All projections as fp8e4 DoubleRow matmuls (contraction 256 = 128x2).
  - S^T per (head, k-tile) as fp8-DR matmul with a stride-0 broadcast pair
    dim (computes 2x the true product; folded into exp scale).
  - Per-key softmax bias folded into V: vaug = 16*exp(bT)*v', plus
    replicated denominator columns (M=64/head: even h [attn|den] at rows
    0:64 of the pair psum, odd h [den|attn] at 64:128) -> den rows 32:96
    contiguous: ONE reciprocal_approx_fast per head pair.
  - exp split between ACT (native Exp) and DVE (custom EXP2_SQ16_ANT:
    quadratic + 4 squarings, one uop, <1e-3 rel err), writing fp8 eT2
    tiles [128,2,512] = the DoubleRow rhs of the AV matmul.
  - gate = 1/(1+exp(-y)) via ACT exp + DVE add1 + reciprocal_approx_fast.
  - engine split: GPSIMD takes xn, xT8 convert, vaug ones, final gate-mult
    (all SBUF-only; GPSIMD has no PSUM port). Single act table (Ln+Exp).
"""

import numpy as np
from contextlib import ExitStack

L, B, F, H, C = 1024, 8, 256, 8, 32
HC = H * C
EPS = 1e-5
N_CORES = 8
P = 128
NLT = L // P   # 8 L-tiles
QSC = 8.0      # q/k fp8 pre-scale (folded into Wq/Wk host-side)
S_SCALE = 0.0013810679320049755   # 1/(2*QSC^2*sqrt(C)): psum -> logits
# EXP2_SQ16 constants: out = exp(x*S_SCALE), fit for |logits| <= 0.8
EXP_C0 = 6.103612520034756e-05
EXP_C1 = 0.7073182017382096
EXP_C2 = 0.4997011001858717
VSC = 16.0     # vaug scale (cancels in normalize)

_COMPILED = {}
_EXP2_OP = None


def _register_exp2():
    """Register the custom DVE op EXP2_SQ16_ANT (idempotent)."""
    global _EXP2_OP
    if _EXP2_OP is not None:
        return _EXP2_OP
    import concourse.dve_ops as dvo
    import concourse.dve_spec as dvs
    from concourse.dve_spec import Src0, C0, C1, C2, Spec, sq
    import concourse.dve_uop as dvu

    for op in dvo.OPS:
        if op.name == "EXP2_SQ16_ANT":
            _EXP2_OP = op
            return op

    _body = sq(sq(sq(sq(sq(Src0 * C0 + C1) + C2))))

    def _ref(in0, in1, c0, c1, c2):
        z = in0.astype(np.float32) * np.float32(c0)
        q = (z + np.float32(c1)) * (z + np.float32(c1)) + np.float32(c2)
        for _ in range(4):
            q = q * q
        return q

    spec = Spec(body=_body, reference=_ref)
    row = dvo._CUSTOM_DVE_ROW_BASE + len(dvo.OPS)
    assert row < 0x20, "custom-DVE opcode rows exhausted"
    shas = {}
    for ver in ("v3", "v4"):
        r = dvu.DveOpSpec(name="EXP2_SQ16_ANT", opcode=row,
                          uops=dvs.lower(spec, ver=ver),
                          rd1_en=dvs._has_src1(spec))
        shas[ver] = r.sha(ver)
    op = dvo.DveOp("EXP2_SQ16_ANT", spec, subdim=False, uops_sha=shas)
    dvo.OPS.append(op)
    dvo.CUSTOM_DVE_SPECS[op.name] = op.spec
    dvo._SUB_OPCODE_FOR_NAME[op.name] = row
    _EXP2_OP = op
    return op


def _build(bv_zero=True, bo_zero=True, bqk_zero=True, dve_exp_every=4):
    import concourse.bacc as bacc
    import concourse.mybir as mybir
    import concourse.tile as tile
    from concourse.dve_ops import RECIP_APPROX_FAST_CONSTS, RECIPROCAL_APPROX_FAST

    exp2op = _register_exp2()

    f32 = mybir.dt.float32
    bf16 = mybir.dt.bfloat16
    fp8 = mybir.dt.float8e4
    u8 = mybir.dt.uint8
    u16 = mybir.dt.uint16
    AF = mybir.ActivationFunctionType
    ALU = mybir.AluOpType
    DR = mybir.MatmulPerfMode.DoubleRow
    RCN = RECIP_APPROX_FAST_CONSTS

    nc = bacc.Bacc("TRN2", target_bir_lowering=False)

    feat_e = nc.dram_tensor("feat", [L, F], f32, kind="ExternalInput")
    wq_e = nc.dram_tensor("wq16", [P, 2, HC], u16, kind="ExternalInput")
    wk_e = nc.dram_tensor("wk16", [P, 2, HC], u16, kind="ExternalInput")
    wg_e = nc.dram_tensor("wg16", [P, 2, HC], u16, kind="ExternalInput")
    wvb_e = nc.dram_tensor("wvb16", [P, 2, HC + H], u16, kind="ExternalInput")
    wo_e = nc.dram_tensor("wo16", [P, 2, F], u16, kind="ExternalInput")
    bq_e = nc.dram_tensor("bq_t", [P, 2], f32, kind="ExternalInput")
    bk_e = nc.dram_tensor("bk_t", [P, 2], f32, kind="ExternalInput")
    nbg_e = nc.dram_tensor("nbg_t", [P, 2], f32, kind="ExternalInput")
    bv_e = nc.dram_tensor("bv_t", [P, 2], f32, kind="ExternalInput")
    bo_e = nc.dram_tensor("bo_b", [P, F], f32, kind="ExternalInput")
    out_e = nc.dram_tensor("out", [L, F], f32, kind="ExternalOutput")

    with tile.TileContext(nc) as tc, ExitStack() as ctx:
        const = ctx.enter_context(tc.tile_pool(name="const", bufs=1))
        main = ctx.enter_context(tc.tile_pool(name="main", bufs=1))
        work = ctx.enter_context(tc.tile_pool(name="work", bufs=4))
        epool = ctx.enter_context(tc.tile_pool(name="epool", bufs=4))
        npool = ctx.enter_context(tc.tile_pool(name="npool", bufs=4))
        opool = ctx.enter_context(tc.tile_pool(name="opool", bufs=4))

        ftp = ctx.enter_context(tc.tile_pool(name="ftp", bufs=1))
        ft = [ftp.tile([P, F], f32, name=f"ft{i}") for i in range(NLT)]
        for i in range(NLT):
            (nc.sync if i % 2 == 0 else nc.gpsimd).dma_start(
                ft[i][:], feat_e.ap()[i * P:(i + 1) * P, :])

        def load(name, ext, shape, dt_):
            t = const.tile(shape, dt_, name=name)
            nc.sync.dma_start(t[:], ext.ap().bitcast(dt_))
            return t

        wq16 = load("wq16_s", wq_e, [P, 2, HC], bf16)
        wk16 = load("wk16_s", wk_e, [P, 2, HC], bf16)
        wvb16 = load("wvb16_s", wvb_e, [P, 2, HC + H], bf16)
        epst = const.tile([P, 1], f32, name="epst")
        nc.vector.memset(epst[:], EPS)
        ln16t = const.tile([P, 1], f32, name="ln16t")
        nc.vector.memset(ln16t[:], float(np.log(VSC)))
        dumt = const.tile([P, 1], f32, name="dumt")
        nc.scalar.activation(dumt[:], epst[:], AF.Exp)

        xT16 = [main.tile([P, L], bf16, name=f"xT16_{j}") for j in range(2)]
        q8 = [main.tile([P, L], fp8, name=f"q8_{j}") for j in range(2)]
        k8 = [main.tile([P, L], fp8, name=f"k8_{j}") for j in range(2)]
        g16 = [main.tile([P, L], bf16, name=f"g16_{j}") for j in range(2)]
        agT = [main.tile([P, L], bf16, name=f"agT{j}") for j in range(2)]
        v16 = main.tile([P, NLT, H, 64], bf16, name="v16")
        ebT = [main.tile([P, H], f32, name=f"ebT{i}") for i in range(NLT)]

        # ================= Stage A: LN + DMA transpose =================
        for i in range(NLT):
            st6 = work.tile([P, 6], f32, tag="st6")
            nc.vector.bn_stats(st6[:], ft[i][:])
            ag = work.tile([P, 4], f32, tag="ag")
            nc.vector.bn_aggr(ag[:, 0:2], st6[:])
            nc.scalar.activation(ag[:, 2:3], ag[:, 1:2], AF.Ln, bias=epst[:])
            nc.scalar.activation(ag[:, 3:4], ag[:, 2:3], AF.Exp, scale=-0.5)
            xn = work.tile([P, F], bf16, tag="xn")
            nc.vector.tensor_scalar(xn[:], ft[i][:], ag[:, 0:1], ag[:, 3:4],
                                    op0=ALU.subtract, op1=ALU.mult)
            ls = slice(i * P, (i + 1) * P)
            for jc in range(2):
                nc.sync.dma_start_transpose(xT16[jc][:, ls],
                                            xn[:, jc * P:(jc + 1) * P])

        wg16 = load("wg16_s", wg_e, [P, 2, HC], bf16)
        wo16 = load("wo16_s", wo_e, [P, 2, F], bf16)
        bq4 = load("bq4_s", bq_e, [P, 2], f32)
        bk4 = load("bk4_s", bk_e, [P, 2], f32)
        nbg = load("nbg_s", nbg_e, [P, 2], f32)
        bvt = load("bvt_s", bv_e, [P, 2], f32)
        bob = load("bob_s", bo_e, [P, F], f32)


        # ================= Stage B: projections =================
        psB_cm = tc.tile_pool(name="psB", bufs=3, space="PSUM")
        psB = psB_cm.__enter__()
        psG_cm = tc.tile_pool(name="psG", bufs=1, space="PSUM")
        psG = psG_cm.__enter__()
        psVp_cm = tc.tile_pool(name="psVp", bufs=2, space="PSUM")
        psVp = psVp_cm.__enter__()

        for jc in range(2):
            cs = slice(jc * P, (jc + 1) * P)
            for m in range(2):
                ms = slice(512 * m, 512 * (m + 1))
                pq = psB.tile([P, 512], f32, tag="pb", name=f"pq{jc}{m}")
                pk = psB.tile([P, 512], f32, tag="pb", name=f"pk{jc}{m}")
                for fc in range(2):
                    nc.tensor.matmul(pq[:], wq16[:, fc, cs],
                                     xT16[fc][:, ms], start=(fc == 0),
                                     stop=(fc == 1))
                    nc.tensor.matmul(pk[:], wk16[:, fc, cs],
                                     xT16[fc][:, ms], start=(fc == 0),
                                     stop=(fc == 1))
                if bqk_zero:
                    if m == 0:
                        nc.scalar.copy(q8[jc][:, ms], pq[:])
                        nc.scalar.copy(k8[jc][:, ms], pk[:])
                    else:
                        nc.vector.tensor_copy(q8[jc][:, ms], pq[:])
                        nc.vector.tensor_copy(k8[jc][:, ms], pk[:])
                elif m == 0:
                    nc.scalar.activation(q8[jc][:, ms], pq[:], AF.Identity,
                                         bias=bq4[:, jc:jc + 1])
                    nc.scalar.activation(k8[jc][:, ms], pk[:], AF.Identity,
                                         bias=bk4[:, jc:jc + 1])
                else:
                    nc.vector.tensor_scalar(q8[jc][:, ms], pq[:],
                                            bq4[:, jc:jc + 1], None,
                                            op0=ALU.add)
                    nc.vector.tensor_scalar(k8[jc][:, ms], pk[:],
                                            bk4[:, jc:jc + 1], None,
                                            op0=ALU.add)
            pg = psG.tile([P, L], f32, tag="pg", name=f"pg{jc}")
            for m in range(2):
                ms = slice(512 * m, 512 * (m + 1))
                for fc in range(2):
                    nc.tensor.matmul(pg[:, ms], wg16[:, fc, cs],
                                     xT16[fc][:, ms], start=(fc == 0),
                                     stop=(fc == 1))
            eg = work.tile([P, L], f32, tag="eg", name=f"eg{jc}")
            nc.scalar.activation(eg[:], pg[:], AF.Exp,
                                 bias=nbg[:, jc:jc + 1], scale=-1.0)
            nc.vector.tensor_scalar(eg[:], eg[:], 1.0, None, op0=ALU.add)
            nc.vector._custom_dve(RECIPROCAL_APPROX_FAST, out=g16[jc][:],
                                  in0=eg[:], s0=RCN["s0"], s1=RCN["s1"],
                                  imm2=RCN["imm2"])

        for i in range(NLT):
            ls = slice(i * P, (i + 1) * P)
            pv = psVp.tile([P, HC + H], f32, tag="pv", name=f"pv{i}")
            for fc in range(2):
                nc.tensor.matmul(pv[:], xT16[fc][:, ls], wvb16[:, fc, :],
                                 start=(fc == 0), stop=(fc == 1))
            # eb = 16*exp(bT): fold the vaug scale into the exp bias
            nc.scalar.activation(ebT[i][:], pv[:, HC:HC + H], AF.Exp,
                                 bias=ln16t[:])
            vv = pv[:, 0:HC].rearrange("p (h c) -> p h c", h=H)
            eb = ebT[i][:].unsqueeze(2).to_broadcast([P, H, 32])
            nc.vector.tensor_tensor(v16[:, i, :, 0:32], vv[:], eb,
                                    op=ALU.mult)
            nc.gpsimd.tensor_copy(v16[:, i, :, 32:64], eb)

        psVp_cm.__exit__(None, None, None)
        psG_cm.__exit__(None, None, None)
        psB_cm.__exit__(None, None, None)

        # ================= Stage C: attention =================
        psS_cm = tc.tile_pool(name="psS", bufs=2, space="PSUM")
        psS = psS_cm.__enter__()
        psV_cm = tc.tile_pool(name="psV", bufs=2, space="PSUM")
        psV = psV_cm.__enter__()

        expctr = 0
        for h in range(H):
            jh, hp = h // 4, slice(32 * (h % 4), 32 * (h % 4) + 32)
            aph = psV.tile([64, L], f32, tag="av", name=f"aph{h}")
            for p in range(4):  # k-tile pairs
                for m in range(2):
                    ms = slice(512 * m, 512 * (m + 1))
                    sp = psS.tile([P, L], f32, tag="sp", name=f"sp{h}{p}{m}")
                    for j in range(2):
                        ks = slice((2 * p + j) * P, (2 * p + j + 1) * P)
                        nc.tensor.matmul(
                            sp[:, 512 * j:512 * (j + 1)],
                            k8[jh][hp, ks].unsqueeze(1)
                                .to_broadcast([32, 2, P]),
                            q8[jh][hp, ms].unsqueeze(1)
                                .to_broadcast([32, 2, 512]),
                            start=True, stop=True, perf_mode=DR,
                            tile_position=(32 * (h % 4), 0))
                    eT = epool.tile([P, 2, 512], bf16, tag="eT",
                                    name=f"eT{h}{p}{m}")
                    spv = sp[:].rearrange("a (b c) -> a b c", b=2)
                    expctr += 1
                    if dve_exp_every and expctr % dve_exp_every == 0 \
                            and expctr <= 56:
                        nc.vector._custom_dve(exp2op, out=eT[:], in0=spv,
                                              s0=EXP_C0, s1=EXP_C1,
                                              imm2=EXP_C2)
                    else:
                        nc.scalar.activation(eT[:], spv, AF.Exp,
                                             scale=S_SCALE)
                    for j in range(2):
                        nc.tensor.matmul(aph[:, ms], v16[:, 2 * p + j, h, :],
                                         eT[:, j, :],
                                         start=(p == 0 and j == 0),
                                         stop=(p == 3 and j == 1))
            # drain: nrm = 1/aph (rows 32:64 are the denominator; rows 0:32
            # give unused junk — cost is free-size only), th = attn*nrm,
            # agT = (th [+bv]) * gate on GPSIMD (SBUF-only)
            nrm = npool.tile([64, L], bf16, tag="nrm", name=f"nrm{h}")
            th = npool.tile([P, L], bf16, tag="th", name=f"th{h}")
            msplits = ([slice(0, 512), slice(512, 1024)] if h == H - 1
                       else [slice(0, L)])
            for dms in msplits:
                nc.vector._custom_dve(RECIPROCAL_APPROX_FAST,
                                      out=nrm[:, dms], in0=aph[:, dms],
                                      s0=RCN["s0"], s1=RCN["s1"],
                                      imm2=RCN["imm2"])
                nc.vector.tensor_tensor(th[hp, dms], aph[0:32, dms],
                                        nrm[32:64, dms], op=ALU.mult)
                nc.gpsimd.tensor_tensor(agT[jh][hp, dms], th[hp, dms],
                                        g16[jh][hp, dms], op=ALU.mult)
                if not bv_zero:
                    nc.vector.scalar_tensor_tensor(
                        agT[jh][hp, dms], g16[jh][hp, dms],
                        bvt[hp, jh:jh + 1], agT[jh][hp, dms],
                        op0=ALU.mult, op1=ALU.add)

        psV_cm.__exit__(None, None, None)

        # ================= Stage D: output projection =================
        for i in range(NLT):
            ls = slice(i * P, (i + 1) * P)
            po = psS.tile([P, F], f32, tag="sp", name=f"po{i}")
            nc.tensor.matmul(po[:], agT[0][:, ls], wo16[:, 0, :],
                             start=True, stop=False)
            nc.tensor.matmul(po[:], agT[1][:, ls], wo16[:, 1, :],
                             start=False, stop=True)
            o = opool.tile([P, F], f32, tag="o", name=f"o{i}")
            if bo_zero:
                if i % 2 == 0:
                    nc.vector.tensor_copy(o[:], po[:])
                else:
                    nc.scalar.copy(o[:], po[:])
            else:
                nc.vector.tensor_tensor(o[:], po[:], bob[:], op=ALU.add)
            nc.sync.dma_start(out_e.ap()[ls, :], o[:])

        psS_cm.__exit__(None, None, None)

    # single activation table (Exp+Ln) to avoid chooser thrash
    import concourse.bacc as bacc_mod
    orig_gat = bacc_mod.get_activation_tables
    AFt = mybir.ActivationFunctionType

    def gat_combined(arch):
        t = orig_gat(arch)
        out = {}
        drop = {AFt.Exp, AFt.Ln}
        for name, funcs in t.items():
            if name == "natural_log_exp_and_others":
                out[name] = funcs
            else:
                out[name] = funcs - drop
        return out

    bacc_mod.get_activation_tables = gat_combined
    try:
        nc.compile()
    finally:
        bacc_mod.get_activation_tables = orig_gat
    return nc


def _prep_inputs(features, ln_g, ln_b, Wq, bq, Wk, bk, Wv, bv, Wb, bb,
                 Wg, bg, Wo, bo):
    import ml_dtypes
    f32 = np.float32
    fp8 = ml_dtypes.float8_e4m3
    bfl = ml_dtypes.bfloat16
    g_ = np.asarray(ln_g, f32)[:, None]
    b_ = np.asarray(ln_b, f32)

    def chunk3(W, scale, dt):
        Wn = (np.asarray(W, f32) * g_ * scale).reshape(2, P, -1)
        return np.ascontiguousarray(Wn.transpose(1, 0, 2)).astype(dt)

    def bchunk(v):
        return np.ascontiguousarray(np.asarray(v, f32).reshape(2, P).T)

    Wq_ = chunk3(Wq, QSC, bfl).view(np.uint16)
    Wk_ = chunk3(Wk, QSC, bfl).view(np.uint16)
    Wg_ = chunk3(Wg, 1.0, bfl).view(np.uint16)
    Wvb = np.concatenate([np.asarray(Wv, f32), np.asarray(Wb, f32)], axis=1)
    Wvb_ = chunk3(Wvb, 1.0, bfl).view(np.uint16)
    Wo_ = np.ascontiguousarray(
        np.asarray(Wo, f32).reshape(2, P, F).transpose(1, 0, 2)
    ).astype(bfl).view(np.uint16)

    bqf = (b_ @ np.asarray(Wq, f32) + np.asarray(bq, f32)) * QSC
    bkf = (b_ @ np.asarray(Wk, f32) + np.asarray(bk, f32)) * QSC
    bgf = b_ @ np.asarray(Wg, f32) + np.asarray(bg, f32)
    bvf = b_ @ np.asarray(Wv, f32) + np.asarray(bv, f32)

    common = {
        "wq16": Wq_, "wk16": Wk_, "wg16": Wg_, "wvb16": Wvb_, "wo16": Wo_,
        "bq_t": bchunk(bqf), "bk_t": bchunk(bkf), "nbg_t": bchunk(-bgf),
        "bv_t": bchunk(bvf),
        "bo_b": np.ascontiguousarray(np.tile(np.asarray(bo, f32), (P, 1))),
    }
    flags = (bool(np.all(bvf == 0.0)), bool(np.all(np.asarray(bo, f32) == 0.0)),
             bool(np.all(bqf == 0.0) and np.all(bkf == 0.0)))
    feats = np.asarray(features, f32)
    in_maps = []
    for b in range(N_CORES):
        m = dict(common)
        m["feat"] = np.ascontiguousarray(feats[:, b, :])
        in_maps.append(m)
    return in_maps, flags


def kernel(**inputs):
    from concourse.bass_utils import run_bass_kernel_spmd

    in_maps, flags = _prep_inputs(**inputs)
    key = ("nc",) + flags
    if key not in _COMPILED:
        bv_zero, bo_zero, bqk_zero = flags
        _COMPILED[key] = _build(bv_zero=bv_zero, bo_zero=bo_zero,
                                bqk_zero=bqk_zero)
        _COMPILED["nc"] = _COMPILED[key]
    nc = _COMPILED[key]
    res = run_bass_kernel_spmd(nc, in_maps, list(range(N_CORES)))
    out = np.stack([res.results[b]["out"] for b in range(N_CORES)], axis=1)
    return np.ascontiguousarray(out.astype(np.float32))


if __name__ == "__main__":
    rng = np.random.default_rng(0)
    ins = {
        "features": rng.standard_normal((L, B, F), dtype=np.float32),
        "ln_g": np.ones(F, np.float32), "ln_b": np.zeros(F, np.float32),
        "Wq": rng.standard_normal((F, HC), dtype=np.float32) * 0.02,
        "bq": np.zeros(HC, np.float32),
        "Wk": rng.standard_normal((F, HC), dtype=np.float32) * 0.02,
        "bk": np.zeros(HC, np.float32),
        "Wv": rng.standard_normal((F, HC), dtype=np.float32) * 0.02,
        "bv": np.zeros(HC, np.float32),
        "Wb": rng.standard_normal((F, H), dtype=np.float32) * 0.02,
        "bb": np.zeros(H, np.float32),
        "Wg": rng.standard_normal((F, HC), dtype=np.float32) * 0.02,
        "bg": np.zeros(HC, np.float32),
        "Wo": rng.standard_normal((HC, F), dtype=np.float32) * 0.02,
        "bo": np.zeros(F, np.float32),
    }
    print(kernel(**ins).shape)
